# revision 22
# baseline (speedup 1.0000x reference)
"""Trainium2 Bass kernel for nn_CTMDNXCell (CTM-style recurrent cell).

Strategy
--------
Pure data parallel over the batch: each of the 8 NeuronCores gets B/8 = 4096
rows.  All state lives in SBUF in *transposed* layout ([feature, batch]) for
the whole 6-step unfold, so HBM traffic is just load-inputs + store-outputs.
All per-core inputs are packed into ONE [128, NCOL_IN] DRAM tensor loaded by a
single DMA (avoids the per-instruction sync-wait limit on early consumers).

Reformulation (validated against the jax reference on host):
  state   w := h - A                         (so the Wf@h bias folds into the
                                              cos() bias, per-partition)
  f_total = sin(z)^2 = (1 - cos(2 z)) / 2
  h'      = h + 2*DT*dh  (speculative no-break)
          => w' = (DT*c + (c1-DT)) ∘ w + (c1-1)*A,   c = cos(2 z),
             c1 = 1 - 2*DT/tau
  beta_t  is closed-form:  beta^(n) = rs^n b0 + (1-rs^n)/(1-rs), so
  1/sqrt(beta) = exp(-0.5*ln(rs^n b0 + k_n))   (2 ACT ops, ln/exp share an
                                                activation-table set)
  pair    = (GL@w + A[il]) ∘ (GR@w + A[ir])   (one-hot gather matmuls on PE)

The global early-exit test (mean|hu| < 0.01 at steps >= 3) is handled
speculatively: the kernel always runs the no-break trajectory and emits
per-step partial sums of |h'-h|; the host reduces them, and in the (never
observed for these inputs) event a break fires, recomputes exactly in numpy.
"""

import os
import sys

import numpy as np

sys.path.insert(0, "/opt/trn_rl_repo")

# ---------------------------------------------------------------- constants
B, INPUT, HIDDEN, NSYNC = 32768, 128, 256, 256
UNFOLDS = 6
DELTA_T = 0.1
THRESH = 0.01
EPS = 1e-6
NCORES = 8
BS = B // NCORES          # 4096 rows per core
P = 128                   # partitions
CT = 512                  # matmul col tile (one PSUM bank)
GRP = 1024                # elementwise group width
NGRP = BS // GRP          # 2
TPG = GRP // CT           # 4 col tiles per group
NORM_STEPS = (3, 4, 5)
NSTAT = 16                # stats columns (12 used)

# packed input block column offsets (per core, [128 x NCOL_IN] fp32)
OFF_X = 0
OFF_W = OFF_X + BS                 # 2 blocks
OFF_AL = OFF_W + 2 * BS
OFF_B0 = OFF_AL + 2 * BS
OFF_WFX = OFF_B0 + 2 * BS          # [128, 256]
OFF_WFH = OFF_WFX + HIDDEN         # 2 blocks of [128, 256]
OFF_WM = OFF_WFH + 2 * HIDDEN
OFF_GL = OFF_WM + 2 * HIDDEN
OFF_GR = OFF_GL + 2 * NSYNC
OFF_CST = OFF_GR + 2 * NSYNC       # [128, 64]
NCST = 64
NCOL_IN = OFF_CST + NCST

# packed output block column offsets (per core, [128 x NCOL_OUT] fp32)
OOF_H = 0                          # 2 blocks
OOF_A = OOF_H + 2 * BS
OOF_B = OOF_A + 2 * BS
OOF_ST = OOF_B + 2 * BS
NCOL_OUT = OOF_ST + NSTAT

_prog_cache = {}
LAST_EXEC_NS = None


def _patch_act_tables():
    import concourse.bacc as bacc
    from concourse import mybir

    if getattr(bacc, "_ctm_act_tables_patched", False):
        return
    orig = bacc.get_activation_tables

    def patched(arch):
        t = orig(arch)
        LN = mybir.ActivationFunctionType.Ln
        EXP = mybir.ActivationFunctionType.Exp
        SIN = mybir.ActivationFunctionType.Sin
        for name, fns in t.items():
            if name != "natural_log_exp_and_others":
                fns.discard(LN)
                fns.discard(EXP)
            if name != "trig_and_small":
                fns.discard(SIN)
        return t

    bacc.get_activation_tables = patched
    bacc._ctm_act_tables_patched = True


def _build(c1: float, repeat: int = 1, hw_loop: int = 0, skip=()):
    import contextlib

    import concourse.bass as bass
    import concourse.bacc as bacc
    import concourse.tile as tile
    from concourse import mybir

    _patch_act_tables()

    f32 = mybir.dt.float32
    AF = mybir.ActivationFunctionType
    OP = mybir.AluOpType

    nc = bacc.Bacc()

    in_d = nc.declare_dram_parameter("in_blk", [P, NCOL_IN], f32, isOutput=False)
    out_d = nc.declare_dram_parameter("out_blk", [P, NCOL_OUT], f32, isOutput=True)

    with tile.TileContext(nc) as tc, contextlib.ExitStack() as ctx:
        persist = ctx.enter_context(tc.tile_pool(name="persist", bufs=1))
        grp = ctx.enter_context(tc.tile_pool(name="grp", bufs=2))
        psg = ctx.enter_context(tc.tile_pool(name="psg", bufs=1, space="PSUM"))
        psz = ctx.enter_context(tc.tile_pool(name="psz", bufs=2, space="PSUM"))

        IN = persist.tile([P, NCOL_IN], f32, tag="IN", name="IN")
        nc.sync.dma_start(out=IN, in_=in_d[:, :])

        x_s = IN[:, OFF_X:OFF_X + BS]
        w_s = [IN[:, OFF_W + b * BS:OFF_W + (b + 1) * BS] for b in range(2)]
        al_s = [IN[:, OFF_AL + b * BS:OFF_AL + (b + 1) * BS] for b in range(2)]
        b0_s = [IN[:, OFF_B0 + b * BS:OFF_B0 + (b + 1) * BS] for b in range(2)]
        wfx_s = IN[:, OFF_WFX:OFF_WFX + HIDDEN]
        wfh_s = [IN[:, OFF_WFH + b * HIDDEN:OFF_WFH + (b + 1) * HIDDEN]
                 for b in range(2)]
        wm_s = [IN[:, OFF_WM + b * HIDDEN:OFF_WM + (b + 1) * HIDDEN]
                for b in range(2)]
        gl_s = [IN[:, OFF_GL + b * NSYNC:OFF_GL + (b + 1) * NSYNC]
                for b in range(2)]
        gr_s = [IN[:, OFF_GR + b * NSYNC:OFF_GR + (b + 1) * NSYNC]
                for b in range(2)]
        cst = IN[:, OFF_CST:OFF_CST + NCST]

        stats = persist.tile([P, NSTAT], f32, tag="stats", name="stats")
        nc.vector.memset(stats, 0.0)

        # Prime each compute engine's vector clock with the input-DMA tick so
        # no later instruction needs a dedicated DMA sync-wait slot (HW allows
        # very few waits per instruction).
        prime = persist.tile([P, 4], f32, tag="prime", name="prime")
        nc.vector.tensor_copy(prime[:, 0:1], IN[:, 0:1])
        nc.scalar.copy(prime[:, 1:2], IN[:, 0:1])
        zp = psz.tile([1, 1], f32, tag="z0", name="zp")
        nc.tensor.matmul(zp, IN[:, 0:1], IN[:, 0:1], start=True, stop=True)

        def C(col):
            return cst[:, col:col + 1]

        RS = lambda b: C(0 + b)
        RSPOW = lambda s, b: C(2 + 2 * s + b)
        KBIAS = lambda s, b: C(14 + 2 * s + b)
        AL = lambda b: C(26 + b)
        AR = lambda b: C(28 + b)
        CB = lambda b: C(30 + b)
        GC = lambda b: C(32 + b)
        AH = lambda b: C(34 + b)

        import contextlib as _ctxlib

        loop_cm = tc.For_i(0, hw_loop, 1) if hw_loop else _ctxlib.nullcontext()
        with loop_cm:
          for rep in range(repeat):
           for s in range(UNFOLDS):
            is_norm = s in NORM_STEPS

            # ---- 1/sqrt(beta) for both groups (contiguous ln/exp) ------
            u_t = {}
            for g in range(NGRP):
                gs = slice(g * GRP, (g + 1) * GRP)
                for b in range(2):
                    u = grp.tile([P, GRP], f32, tag=f"u{b}", name=f"u{b}", bufs=3)
                    nc.scalar.activation(u, b0_s[b][:, gs], AF.Ln,
                                         bias=KBIAS(s, b), scale=RSPOW(s, b))
                    u_t[(g, b)] = u
            for g in range(NGRP):
                for b in range(2):
                    u = u_t[(g, b)]
                    nc.scalar.activation(u, u, AF.Exp, scale=-0.5)

            for g in range(NGRP):
                gs = slice(g * GRP, (g + 1) * GRP)

                # ---- z x/w-part matmuls first (sync-independent PE work)
                z_t = {}
                for t in range(TPG):
                    ts_ = slice((g * TPG + t) * CT, (g * TPG + t + 1) * CT)
                    for h2 in range(2):
                        z = psz.tile([P, CT], f32, tag=f"z{h2}", name=f"z{h2}")
                        nc.tensor.matmul(z, wfx_s[:, h2 * P:(h2 + 1) * P],
                                         x_s[:, ts_], start=True, stop=False)
                        for kb in range(2):
                            nc.tensor.matmul(z, wfh_s[kb][:, h2 * P:(h2 + 1) * P],
                                             w_s[kb][:, ts_], start=False,
                                             stop=False)
                        z_t[(t, h2)] = z

                # ---- gathers + pair -------------------------------------
                e_pair = [grp.tile([P, GRP], f32, tag=f"ea{j}", name=f"ea{j}", bufs=3)
                          for j in range(2)]
                for t in range(TPG) if "gather" not in skip else []:
                    ts_ = slice((g * TPG + t) * CT, (g * TPG + t + 1) * CT)
                    tl = slice(t * CT, (t + 1) * CT)
                    for j in range(2):
                        pl = psg.tile([P, CT], f32, tag="pl", name="pl", bufs=2)
                        pr = psg.tile([P, CT], f32, tag="pr", name="pr", bufs=2)
                        for kb in range(2):
                            nc.tensor.matmul(pl, gl_s[kb][:, j * P:(j + 1) * P],
                                             w_s[kb][:, ts_],
                                             start=(kb == 0), stop=(kb == 1))
                        for kb in range(2):
                            nc.tensor.matmul(pr, gr_s[kb][:, j * P:(j + 1) * P],
                                             w_s[kb][:, ts_],
                                             start=(kb == 0), stop=(kb == 1))
                        nc.scalar.activation(e_pair[j][:, tl], pr, AF.Identity,
                                             bias=AR(j))
                        nc.vector.scalar_tensor_tensor(
                            e_pair[j][:, tl], pl, AL(j), e_pair[j][:, tl],
                            OP.add, OP.mult)

                # ---- alpha + sync ---------------------------------------
                sync_t = []
                for b in range(2) if "alpha" not in skip else []:
                    nc.vector.scalar_tensor_tensor(
                        al_s[b][:, gs], al_s[b][:, gs], RS(b), e_pair[b],
                        OP.mult, OP.add)
                    sy = grp.tile([P, GRP], f32, tag=f"sv{b}", name=f"sv{b}",
                                  bufs=2)
                    nc.vector.tensor_mul(sy, al_s[b][:, gs], u_t[(g, b)])
                    sync_t.append(sy)

                # ---- z matmuls + c = cos(2z+cb) -------------------------
                if "alpha" in skip:
                    sync_t = u_t[(g, 0)], u_t[(g, 1)]
                e_c = [grp.tile([P, GRP], f32, tag=f"ea{j}", name=f"ea{j}", bufs=3)
                       for j in range(2)]
                for t in range(TPG) if "z" not in skip else []:
                    ts_ = slice((g * TPG + t) * CT, (g * TPG + t + 1) * CT)
                    tl = slice(t * CT, (t + 1) * CT)
                    for h2 in range(2):
                        z = z_t[(t, h2)]
                        for kb in range(2):
                            nc.tensor.matmul(z, wm_s[kb][:, h2 * P:(h2 + 1) * P],
                                             sync_t[kb][:, tl], start=False,
                                             stop=(kb == 1))
                        # xe = 2z + cb  (PSUM evict on ACT)
                        nc.scalar.activation(e_c[h2][:, tl], z, AF.Identity,
                                             bias=CB(h2))

                # ---- range-reduce + sin:  c = sin(xe - 2*pi*round(xe/2pi))
                MAGIC = 12582912.0  # 1.5 * 2**23
                for b in range(2) if "rr" not in skip else []:
                    k = grp.tile([P, GRP], f32, tag=f"sv{b}", name=f"sv{b}",
                                 bufs=2)
                    nc.vector.tensor_scalar(k, e_c[b], float(1.0 / (2 * np.pi)),
                                            MAGIC, OP.mult, OP.add)
                    nc.vector.tensor_scalar(k, k, -MAGIC, None, OP.add)
                    nc.vector.scalar_tensor_tensor(
                        e_c[b], k, float(-2.0 * np.pi), e_c[b], OP.mult, OP.add)
                    nc.scalar.activation(e_c[b], e_c[b], AF.Sin, bias=C(36))

                # ---- h-chain --------------------------------------------
                for b in range(2) if "h" not in skip else []:
                    v = grp.tile([P, GRP], f32, tag=f"sv{b}", name=f"sv{b}",
                                 bufs=2)
                    # v = DT*c + (c1-DT) ; v *= w
                    nc.vector.tensor_scalar(v, e_c[b], float(DELTA_T),
                                            float(c1 - DELTA_T), OP.mult, OP.add)
                    nc.vector.tensor_mul(v, v, w_s[b][:, gs])
                    if not is_norm:
                        # w' = v + (c1-1)A
                        nc.vector.tensor_scalar(w_s[b][:, gs], v, GC(b), None,
                                                OP.add)
                    else:
                        d = grp.tile([P, GRP], f32, tag=f"u{b}", name=f"u{b}", bufs=3)
                        # d = (v + gC) - w  == h' - h
                        nc.vector.scalar_tensor_tensor(
                            d, v, GC(b), w_s[b][:, gs], OP.add, OP.subtract)
                        # w' = w + d
                        nc.vector.tensor_add(w_s[b][:, gs], w_s[b][:, gs], d)
                        # stats[:, col] = sum(|d|)
                        col = (s - 3) * 4 + g * 2 + b
                        nc.scalar.activation(d, d, AF.Abs,
                                             accum_out=stats[:, col:col + 1])

        # ---------------- outputs ---------------------------------------
        for b in range(2):
            for g in range(NGRP):
                gs = slice(g * GRP, (g + 1) * GRP)
                ho = grp.tile([P, GRP], f32, tag=f"ea{b}", name=f"ea{b}", bufs=3)
                nc.vector.tensor_scalar(ho, w_s[b][:, gs], AH(b), None, OP.add)
                nc.sync.dma_start(
                    out=out_d[:, OOF_H + b * BS + g * GRP:
                              OOF_H + b * BS + (g + 1) * GRP], in_=ho)
                bo = grp.tile([P, GRP], f32, tag=f"u{b}", name=f"u{b}", bufs=3)
                nc.scalar.activation(bo, b0_s[b][:, gs], AF.Identity,
                                     bias=KBIAS(5, b), scale=RSPOW(5, b))
                nc.sync.dma_start(
                    out=out_d[:, OOF_B + b * BS + g * GRP:
                              OOF_B + b * BS + (g + 1) * GRP], in_=bo)
            nc.sync.dma_start(
                out=out_d[:, OOF_A + b * BS:OOF_A + (b + 1) * BS], in_=al_s[b])
        nc.sync.dma_start(out=out_d[:, OOF_ST:OOF_ST + NSTAT], in_=stats)

    nc.finalize()
    return nc


# ---------------------------------------------------------------- host side

def _softplus64(x):
    x = np.float64(x)
    return np.log1p(np.exp(-np.abs(x))) + np.maximum(x, 0.0)


def _sigmoid64(x):
    return 1.0 / (1.0 + np.exp(-x.astype(np.float64)))


def _host_prep(inputs):
    x = np.asarray(inputs["x"], np.float32)
    h = np.asarray(inputs["h"], np.float32)
    al0 = np.asarray(inputs["mem_alpha"], np.float32)
    b0 = np.asarray(inputs["mem_beta"], np.float32)
    Wf = np.asarray(inputs["Wf"], np.float32)
    bf = np.asarray(inputs["bf"], np.float32)
    tau_param = np.asarray(inputs["tau_param"], np.float32)
    r = np.asarray(inputs["r_param"], np.float32)
    A = np.asarray(inputs["A_param"], np.float32)
    Wm = np.asarray(inputs["Wm"], np.float32)
    il = np.asarray(inputs["idx_left"], np.int64)
    ir = np.asarray(inputs["idx_right"], np.int64)

    tau = _softplus64(tau_param)
    inv_tau = 1.0 / tau
    c1 = float(1.0 - 2.0 * DELTA_T * inv_tau)

    rs64 = _sigmoid64(r)
    rs = rs64.astype(np.float32)
    rspow = [(rs64 ** (s + 1)).astype(np.float32) for s in range(UNFOLDS)]
    kbias = [((1.0 - rs64 ** (s + 1)) / (1.0 - rs64)).astype(np.float32)
             for s in range(UNFOLDS)]

    Al = A[il].astype(np.float32)
    Ar = A[ir].astype(np.float32)
    zb = (bf.astype(np.float64)
          + Wf[:, INPUT:].astype(np.float64) @ A.astype(np.float64))
    cb = (np.pi / 2.0 + 2.0 * zb).astype(np.float32)
    gC = ((c1 - 1.0) * A.astype(np.float64)).astype(np.float32)

    cst = np.zeros((P, NCST), np.float32)

    def put(col, vec256):
        v = np.asarray(vec256, np.float32).reshape(2, P)
        cst[:, col] = v[0]
        cst[:, col + 1] = v[1]

    put(0, rs)
    for s in range(UNFOLDS):
        put(2 + 2 * s, rspow[s])
        put(14 + 2 * s, kbias[s])
    put(26, Al)
    put(28, Ar)
    put(30, cb)
    put(32, gC)
    put(34, A)
    cst[:, 36] = np.float32(0.0)

    GLT = np.zeros((HIDDEN, NSYNC), np.float32)
    GLT[il, np.arange(NSYNC)] = 1.0
    GRT = np.zeros((HIDDEN, NSYNC), np.float32)
    GRT[ir, np.arange(NSYNC)] = 1.0

    def two_block(m256xN):
        # [256, N] -> [128, 2N] (block0 | block1)
        return np.concatenate([m256xN[:P], m256xN[P:]], axis=1)

    wblk = np.empty((P, 2 * HIDDEN + 2 * HIDDEN + 2 * NSYNC + 2 * NSYNC + HIDDEN),
                    np.float32)
    # order: wfx | wfh | wm | gl | gr  (matching OFF_* layout)
    # weights doubled so PSUM holds 2*z directly (cos(2z) argument)
    wfx_t = np.ascontiguousarray(2.0 * Wf[:, :INPUT].T)        # [128, 256]
    wfh_t = two_block(np.ascontiguousarray(2.0 * Wf[:, INPUT:].T))   # [128, 512]
    wm_t = two_block(np.ascontiguousarray(2.0 * Wm.T))
    gl_t = two_block(GLT)
    gr_t = two_block(GRT)
    wtail = np.concatenate([wfx_t, wfh_t, wm_t, gl_t, gr_t, cst], axis=1)

    w0 = h - A[None, :]
    in_maps = []
    for c in range(NCORES):
        rows = slice(c * BS, (c + 1) * BS)
        blk = np.concatenate([
            x[rows].T,                       # [128, 4096]
            two_block(w0[rows].T),           # [128, 8192]
            two_block(al0[rows].T),
            two_block(b0[rows].T),
            wtail,
        ], axis=1).astype(np.float32)
        blk = np.ascontiguousarray(blk)
        assert blk.shape == (P, NCOL_IN), blk.shape
        in_maps.append({"in_blk": blk})

    return in_maps, c1, A


def _numpy_fallback(inputs):
    """Exact (float32 numpy) replica of the reference semantics, used only if
    the early-exit actually fires (never observed for the shipped inputs)."""
    f32 = np.float32
    x = np.asarray(inputs["x"], f32)
    h = np.asarray(inputs["h"], f32).copy()
    alpha = np.asarray(inputs["mem_alpha"], f32).copy()
    beta = np.asarray(inputs["mem_beta"], f32).copy()
    Wf = np.asarray(inputs["Wf"], f32)
    bf = np.asarray(inputs["bf"], f32)
    tau = f32(_softplus64(np.asarray(inputs["tau_param"], f32)))
    inv_tau = f32(1.0) / tau
    rs = _sigmoid64(np.asarray(inputs["r_param"], f32)).astype(f32)[None, :]
    A = np.asarray(inputs["A_param"], f32)[None, :]
    Wm = np.asarray(inputs["Wm"], f32)
    il = np.asarray(inputs["idx_left"], np.int64)
    ir = np.asarray(inputs["idx_right"], np.int64)

    done = False
    last = 0
    for step in range(UNFOLDS):
        base = np.concatenate([x, h], axis=-1) @ Wf.T + bf
        pair = h[:, il] * h[:, ir]
        new_alpha = rs * alpha + pair
        new_beta = rs * beta + f32(1.0)
        sync = new_alpha / (np.sqrt(new_beta) + f32(EPS))
        mem_drive = sync @ Wm.T
        f_total = np.sin(base + mem_drive).astype(f32) ** 2
        dh = -h * (inv_tau + f_total) + A * f_total
        hu = f32(DELTA_T) * dh
        norm = np.abs(hu).mean()
        brk = (step >= 3) and (norm < THRESH)
        h_new = (h + hu) if brk else (h + f32(2.0) * hu)
        if not done:
            h, alpha, beta = h_new.astype(f32), new_alpha, new_beta
            last = step
        if brk and not done:
            done = True
    return h, alpha, beta, np.int32(last)


def kernel(**inputs):
    global LAST_EXEC_NS
    from concourse.bass_utils import run_bass_kernel_spmd

    in_maps, c1, A = _host_prep(inputs)

    key = round(float(c1), 12)  # c1 is baked into the program as immediates
    if key not in _prog_cache:
        _prog_cache[key] = _build(c1)
    nc = _prog_cache[key]

    res = run_bass_kernel_spmd(nc, in_maps, list(range(NCORES)))
    LAST_EXEC_NS = res.exec_time_ns
    results = res.results

    h_full = np.empty((B, HIDDEN), np.float32)
    a_full = np.empty((B, NSYNC), np.float32)
    b_full = np.empty((B, NSYNC), np.float32)
    norm_sums = np.zeros(UNFOLDS, np.float64)
    for c in range(NCORES):
        rows = slice(c * BS, (c + 1) * BS)
        ob = results[c]["out_blk"]

        def un_two_block(seg):  # [128, 2*BS] -> [BS, 256]
            return np.concatenate([seg[:, :BS], seg[:, BS:]], axis=0).T

        h_full[rows] = un_two_block(ob[:, OOF_H:OOF_H + 2 * BS])
        a_full[rows] = un_two_block(ob[:, OOF_A:OOF_A + 2 * BS])
        b_full[rows] = un_two_block(ob[:, OOF_B:OOF_B + 2 * BS])
        st = ob[:, OOF_ST:OOF_ST + NSTAT].astype(np.float64)
        for s in NORM_STEPS:
            cols = slice((s - 3) * 4, (s - 3) * 4 + 4)
            norm_sums[s] += st[:, cols].sum()

    denom = 2.0 * B * HIDDEN  # |hu| = |h'-h| / 2
    brk_step = None
    for s in NORM_STEPS:
        if norm_sums[s] / denom < THRESH:
            brk_step = s
            break

    if brk_step is not None:
        return _numpy_fallback(inputs)

    return h_full, a_full, b_full, np.int32(UNFOLDS - 1)
